# revision 9
# baseline (speedup 1.0000x reference)
"""GQA kernel for trn2: B=2, L=2048, D=2048, Hq=32, Hkv=8, dh=64.

Sharding: 1 KV head (= 4 contiguous Q heads) per core; Wq/Wk/Wv
column-sharded by head, Wo row-sharded; partial outputs summed on host.

Layout trick: x is transposed on the host (xT: [D, B*L]) so every
on-device matmul has its contraction dim on partitions without any
on-device transposes:
  Q^T[dq, l]  = (Wq_tile).T @ xT        (lhsT=Wq, rhs=xT)
  K^T[dh, l]  = (Wk_tile).T @ xT
  V[l, dh]    = (xT_tile).T @ Wv        (lhsT=xT, rhs=Wv)
  S^T[k, q]   = (K^T_tile).T @ Q^T      (lhsT=K^T, rhs=Q^T)   contract dh=64
  E           = exp(S^T / 8)            (ScalarE, PSUM->SBUF)
  U[0:65, q]  = [V|1].T @ E             (lhsT=V_aug, rhs=E)   contract Lk
                row 64 of U = softmax denominator (ones column trick)
  attnT       = U[:64] * bcast(1/U[64]) (DVE recip + K=1 matmul bcast + mul)
  out[l, :]  += (attnT_tile).T @ Wo     (lhsT=attnT, rhs=Wo)
"""

import ml_dtypes
import numpy as np

import concourse.bass as bass
import concourse.bacc as bacc
import concourse.mybir as mybir
from concourse.tile import TileContext, add_dep_helper
from concourse.bass_utils import run_bass_kernel_spmd

B, L, D = 2, 2048, 2048
HQ, HKV, DH = 32, 8, 64
GQ = HQ // HKV            # 4 q heads per core
DQ = GQ * DH              # 256
BL = B * L                # 4096
P = 128
NB = 512                  # free-dim block
KD = D // P               # 16 contraction tiles over D
LT = L // P               # 16 Lk tiles per batch
NBLK = L // NB            # 4 Lq blocks per batch
SCALE = 1.0 / 8.0         # 1/sqrt(dh)

F32 = mybir.dt.float32
BF16 = mybir.dt.bfloat16
AF = mybir.ActivationFunctionType

_CACHED = {}


def _pe_sync(nc, producers, reason):
    # Hoist multi-source waits onto a PE nop: the self-loading f32r matmul
    # (S3_LW) can only carry a single sync wait in walrus codegen.
    if not producers:
        return
    nop = nc.tensor.nop(nofuse=True, hint="sponge")
    for p in producers:
        add_dep_helper(nop.ins, p.ins, reason=reason)


def build_nc():
    nc = bacc.Bacc()
    xT = nc.declare_dram_parameter("xT", [D, BL], BF16, isOutput=False)
    wq = nc.declare_dram_parameter("wq", [D, DQ], BF16, isOutput=False)
    wk = nc.declare_dram_parameter("wk", [D, 2 * DH], BF16, isOutput=False)
    wv = nc.declare_dram_parameter("wv", [D, DH], BF16, isOutput=False)
    wo = nc.declare_dram_parameter("wo", [DQ, D], BF16, isOutput=False)
    out = nc.declare_dram_parameter("out", [BL, D], F32, isOutput=True)

    with TileContext(nc) as tc:
        with (
            tc.tile_pool(name="wpool", bufs=1) as wpool,
            tc.tile_pool(name="xpool", bufs=3) as xpool,
            tc.tile_pool(name="qtpool", bufs=3) as qtpool,
            tc.tile_pool(name="ktpool", bufs=2) as ktpool,
            tc.tile_pool(name="vpool", bufs=34) as vpool,
            tc.tile_pool(name="epool", bufs=20) as epool,
            tc.tile_pool(name="atpool", bufs=2) as atpool,
            tc.tile_pool(name="opool", bufs=3) as opool,
            tc.tile_pool(name="bcpool", bufs=2) as bcpool,
            tc.tile_pool(name="rpool", bufs=4) as rpool,
            tc.tile_pool(name="psA", bufs=2, space="PSUM") as psA,
            tc.tile_pool(name="psS", bufs=4, space="PSUM") as psS,
            tc.tile_pool(name="psU", bufs=2, space="PSUM") as psU,
        ):
            # ---- persistent weights ----
            wdmas = []
            wq_sb = wpool.tile([P, KD, DQ], BF16, tag="wq")
            wdmas.append(nc.sync.dma_start(out=wq_sb, in_=wq.rearrange("(k p) m -> p k m", p=P)))
            wk_sb = wpool.tile([P, KD, 2 * DH], BF16, tag="wk")
            wdmas.append(nc.sync.dma_start(out=wk_sb, in_=wk.rearrange("(k p) m -> p k m", p=P)))
            wv_sb = wpool.tile([P, KD, DH], BF16, tag="wv")
            wdmas.append(nc.sync.dma_start(out=wv_sb, in_=wv.rearrange("(k p) m -> p k m", p=P)))
            wo_sb = [wpool.tile([P, D], BF16, tag=f"wo{t}", name=f"wo_sb{t}") for t in range(2)]
            for t in range(2):
                wdmas.append(nc.sync.dma_start(out=wo_sb[t], in_=wo[t * P : (t + 1) * P, :]))
            ones_sb = wpool.tile([1, DH], BF16, tag="ones")
            nc.vector.memset(ones_sb, 1.0)

            for b in range(B):
                # ---------- phase A: projections for batch b ----------
                qt_sb = [qtpool.tile([P, L], BF16, tag="qt", name=f"qt_sb{t}") for t in range(2)]
                kt_sb = ktpool.tile([P, L], BF16, tag="kt")
                v_sb = [vpool.tile([P, DH + 1], BF16, tag="v", name=f"v_sb{k}") for k in range(LT)]
                acopies = []

                for c in range(NBLK):
                    c0 = b * L + c * NB  # column offset in BL
                    xt_all = xpool.tile([P, KD, NB], BF16, tag="xt")
                    xdma = nc.sync.dma_start(
                        out=xt_all,
                        in_=xT.rearrange("(k p) n -> p k n", p=P)[:, :, c0 : c0 + NB],
                    )

                    # Q^T (two 128-row dq tiles)
                    for t in range(2):
                        q_ps = psA.tile([P, NB], F32, tag="acc")
                        for k in range(KD):
                            nc.tensor.matmul(
                                q_ps,
                                lhsT=wq_sb[:, k, t * P : (t + 1) * P],
                                rhs=xt_all[:, k, :],
                                start=(k == 0),
                                stop=(k == KD - 1),
                            )
                        acopies.append(nc.vector.tensor_copy(
                            qt_sb[t][:, c * NB : (c + 1) * NB], q_ps
                        ))
                    # K^T
                    k_ps = psA.tile([P, NB], F32, tag="acc")
                    for k in range(KD):
                        nc.tensor.matmul(
                            k_ps,
                            lhsT=wk_sb[:, k, :],
                            rhs=xt_all[:, k, :],
                            start=(k == 0),
                            stop=(k == KD - 1),
                        )
                    acopies.append(nc.vector.tensor_copy(kt_sb[:, c * NB : (c + 1) * NB], k_ps))
                    # V (natural, Lk-major) + ones column
                    for j in range(NB // P):
                        lk = c * (NB // P) + j
                        v_ps = psA.tile([P, DH], F32, tag="acc")
                        for k in range(KD):
                            nc.tensor.matmul(
                                v_ps,
                                lhsT=xt_all[:, k, j * P : (j + 1) * P],
                                rhs=wv_sb[:, k, :],
                                start=(k == 0),
                                stop=(k == KD - 1),
                            )
                        acopies.append(nc.vector.tensor_copy(v_sb[lk][:, :DH], v_ps))
                        acopies.append(nc.vector.memset(v_sb[lk][:, DH : DH + 1], 1.0))

                # ---------- phases B+C per Lq block ----------
                for c in range(NBLK):
                    at_sb = [atpool.tile([P, NB], BF16, tag="at", name=f"at_sb{t}") for t in range(2)]
                    at_producers = []
                    for g in range(GQ):
                        qg = qt_sb[g // 2][
                            (g % 2) * DH : (g % 2) * DH + DH, c * NB : (c + 1) * NB
                        ]
                        # S^T tiles + exp; interleave PV to keep PE/ACT in step
                        e_sb = []
                        sT_live = []
                        u_ps = psU.tile([P, NB], F32, tag="u")

                        h0 = (g % 2) * DH

                        def qk_step(k):
                            sT = psS.tile([P, NB], F32, tag="sT")
                            nc.tensor.matmul(
                                sT,
                                lhsT=kt_sb[h0 : h0 + DH, k * P : (k + 1) * P],
                                rhs=qg,
                                start=True,
                                stop=True,
                            )
                            e = epool.tile([P, NB], BF16, tag="e")
                            nc.scalar.activation(e, sT, AF.Exp, scale=SCALE)
                            e_sb.append(e)

                        def pv_step(k):
                            nc.tensor.matmul(
                                u_ps[: DH + 1, :],
                                lhsT=v_sb[c * 0 + k][:, :],
                                rhs=e_sb[k],
                                start=(k == 0),
                                stop=(k == LT - 1),
                            )

                        for k in range(4):
                            qk_step(k)
                        for k in range(4, LT):
                            qk_step(k)
                            pv_step(k - 4)
                        for k in range(LT - 4, LT):
                            pv_step(k)

                        # normalize: attnT = U[:64] * bcast(1 / U[64])
                        recip = rpool.tile([1, NB], BF16, tag="r")
                        with nc.allow_low_precision(reason="f32r is fp32-width"):
                            nc.vector.reciprocal(recip, u_ps[DH : DH + 1, :])
                        bc_ps = psS.tile([DH, NB], F32, tag="sT")
                        nc.tensor.matmul(
                            bc_ps, lhsT=ones_sb, rhs=recip, start=True, stop=True
                        )
                        bc_sb = bcpool.tile([DH, NB], F32, tag="bc")
                        nc.vector.tensor_copy(bc_sb, bc_ps)
                        if g % 2 == 0:
                            at_producers.append(nc.vector.tensor_mul(
                                at_sb[g // 2][:DH, :], u_ps[:DH, :], bc_sb
                            ))
                        else:
                            at_tmp = rpool.tile([DH, NB], BF16, tag="at_tmp")
                            nc.vector.tensor_mul(at_tmp, u_ps[:DH, :], bc_sb)
                            at_producers.append(nc.sync.dma_start(
                                out=at_sb[g // 2][DH : 2 * DH, :], in_=at_tmp
                            ))

                    # ---- phase C: O-projection for this Lq block ----
                    for lt in range(NB // P):
                        row0 = b * L + c * NB + lt * P
                        for nb in range(D // NB):
                            o_ps = psA.tile([P, NB], F32, tag="acc")
                            for t in range(2):
                                nc.tensor.matmul(
                                    o_ps,
                                    lhsT=at_sb[t][:, lt * P : (lt + 1) * P],
                                    rhs=wo_sb[t][:, nb * NB : (nb + 1) * NB],
                                    start=(t == 0),
                                    stop=(t == 1),
                                )
                            o_sb = opool.tile([P, NB], F32, tag="o")
                            nc.vector.tensor_copy(o_sb, o_ps)
                            nc.sync.dma_start(
                                out=out[row0 : row0 + P, nb * NB : (nb + 1) * NB],
                                in_=o_sb,
                            )
    nc.compile()
    return nc


def kernel(x, Wq, Wk, Wv, Wo, trace=False):
    x = np.ascontiguousarray(np.asarray(x, dtype=np.float32))
    Wq = np.asarray(Wq, dtype=np.float32)
    Wk = np.asarray(Wk, dtype=np.float32)
    Wv = np.asarray(Wv, dtype=np.float32)
    Wo = np.asarray(Wo, dtype=np.float32)

    xT = np.ascontiguousarray(x.reshape(BL, D).T.astype(ml_dtypes.bfloat16))  # [D, BL]
    Wq = Wq.astype(ml_dtypes.bfloat16)
    Wk = Wk.astype(ml_dtypes.bfloat16)
    Wv = Wv.astype(ml_dtypes.bfloat16)
    Wo = Wo.astype(ml_dtypes.bfloat16)

    in_maps = []
    for i in range(HKV):
        qs = slice(i * DQ, (i + 1) * DQ)
        ks = slice(i * DH, (i + 1) * DH)
        in_maps.append(
            {
                "xT": xT,
                "wq": np.ascontiguousarray(Wq[:, qs]),
                "wk": np.ascontiguousarray(np.concatenate([Wk[:, ks], Wk[:, ks]], axis=1)),
                "wv": np.ascontiguousarray(Wv[:, ks]),
                "wo": np.ascontiguousarray(Wo[qs, :]),
            }
        )

    if "nc" not in _CACHED:
        _CACHED["nc"] = build_nc()
    nc = _CACHED["nc"]

    res = run_bass_kernel_spmd(nc, in_maps, list(range(HKV)), trace=trace)
    acc = np.zeros((BL, D), dtype=np.float32)
    for r in res.results:
        acc += r["out"]
    if trace:
        kernel.last_exec_time_ns = res.exec_time_ns
        kernel.last_results = res
    return acc.reshape(B, L, D)
